# revision 1
# baseline (speedup 1.0000x reference)
"""Trainium2 Bass kernel for CascadeClassifierGNN (3-layer GCN + BN + ReLU,
global mean pool, 2-layer MLP head), sharded across 8 NeuronCores.

Sharding strategy: nodes and their incident (by-destination) edges are
partitioned across the 8 cores; node ids are relabeled per-core by in-degree
so the per-destination ELL gather structure has minimal padding. Per layer,
each core computes its shard of the gather table v = dinv * r (layer 1:
u1 = dinv * (x @ W1')), an AllGather replicates the full table into every
core's HBM, and each core runs indirect-DMA gathers (128 rows x 256B per op)
+ DVE tree reductions to form neighborhood sums for its own destinations.
BatchNorm is folded into the layer weights on the host (W' = W * A, bias B).
The global mean pool uses one-hot selection matmuls accumulated in PSUM
followed by a tiny AllReduce; the MLP head runs replicated on every core.
"""

import math
import os

import numpy as np

import concourse.bacc as bacc
import concourse.mybir as mybir
import concourse.tile as tile
from concourse import bass_utils
from concourse.bass import IndirectOffsetOnAxis
from concourse.masks import make_identity

F32 = mybir.dt.float32
I32 = mybir.dt.int32
ALU = mybir.AluOpType

# Problem configuration (hardcoded per contest contract).
N = 100000
E = 1600000
F_IN = 10
H = 64
B = 128
C = 3
EPS = 1e-5
M = 8           # cores
P = 128         # partitions
SUPER = 4       # dst tiles per super-tile (gather/reduce granularity)
K_CHUNK = 32    # max ELL slots gathered per chunk

TRACE = os.environ.get("GNN_TRACE", "0") == "1"
LAST_EXEC_NS = None


def _fold_bn(Wl, bl, gl, bel, ml, vl):
    A = (np.asarray(gl, np.float32)
         / np.sqrt(np.asarray(vl, np.float32) + np.float32(EPS)))
    Wp = (np.asarray(Wl, np.float32) * A[None, :]).astype(np.float32)
    Bv = ((np.asarray(bl, np.float32) - np.asarray(ml, np.float32)) * A
          + np.asarray(bel, np.float32)).astype(np.float32)
    return Wp, Bv


def preprocess(x, edge_index, batch, W1, b1, g1, be1, m1, v1,
               W2, b2, g2, be2, m2, v2, W3, b3, g3, be3, m3, v3,
               fw1, fb1, fw2, fb2):
    x = np.asarray(x, dtype=np.float32)
    src = np.asarray(edge_index[0], dtype=np.int64)
    dst = np.asarray(edge_index[1], dtype=np.int64)
    batch = np.asarray(batch, dtype=np.int64)

    assert N % M == 0
    NS = N // M
    T = math.ceil((NS + 1) / (P * SUPER)) * SUPER
    NSP = T * P
    GSUP = T // SUPER
    NT = M * NSP

    deg = (np.bincount(dst, minlength=N) + 1.0).astype(np.float32)
    dinv = (1.0 / np.sqrt(deg)).astype(np.float32)

    relabel = np.empty(N, dtype=np.int64)
    perms = []
    for c in range(M):
        lo = c * NS
        order = np.argsort(-(deg[lo:lo + NS]), kind="stable")
        perms.append(order)
        relabel[lo + order] = c * NSP + np.arange(NS)
    new_src = relabel[src].astype(np.int32)

    owner = dst // NS
    local_rank = relabel[dst] - owner * NSP

    per_core = []
    Kg = np.zeros(GSUP, dtype=np.int64)
    for c in range(M):
        mask = owner == c
        e_rank = local_rank[mask]
        e_src = new_src[mask]
        cnt = np.bincount(e_rank, minlength=NSP)
        per_core.append((e_rank, e_src, cnt))
        cg = cnt.reshape(GSUP, SUPER * P).max(axis=1)
        Kg = np.maximum(Kg, cg)
    Kg = Kg.astype(np.int64)

    goff = np.zeros(GSUP + 1, dtype=np.int64)
    goff[1:] = np.cumsum(P * SUPER * Kg)
    TOT = int(goff[-1])

    dead_rows = (np.arange(M) * NSP + NSP - 1).astype(np.int32)

    ell = np.empty((M, TOT), dtype=np.int32)
    for c in range(M):
        e_rank, e_src, cnt = per_core[c]
        arr = np.full(TOT, dead_rows[c], dtype=np.int32)
        order = np.argsort(e_rank, kind="stable")
        er = e_rank[order]
        es = e_src[order]
        start = np.zeros(NSP, dtype=np.int64)
        start[1:] = np.cumsum(cnt)[:-1]
        slot = np.arange(er.shape[0], dtype=np.int64) - start[er]
        t = er // P
        p = er % P
        g = t // SUPER
        b_ = t % SUPER
        pos = goff[g] + p * (Kg[g] * SUPER) + slot * SUPER + b_
        arr[pos] = es
        ell[c] = arr

    xs = np.zeros((M, NSP, F_IN), dtype=np.float32)
    dinv_sh = np.zeros((M, NSP), dtype=np.float32)
    batch_sh = np.full((M, NSP), -1, dtype=np.int32)
    for c in range(M):
        lo = c * NS
        order = perms[c]
        xs[c, :NS] = x[lo + order]
        dinv_sh[c, :NS] = dinv[lo + order]
        batch_sh[c, :NS] = batch[lo + order]

    dv = dinv_sh.reshape(M, T, P).transpose(0, 2, 1)
    dinv_bc = np.repeat(dv[:, :, :, None], H, axis=3).reshape(M, P, T * H)
    dinv_bc = np.ascontiguousarray(dinv_bc, dtype=np.float32)

    counts = np.bincount(batch, minlength=B).astype(np.float32)
    cinv = (1.0 / np.maximum(counts, 1.0)).astype(np.float32)

    W1p, B1 = _fold_bn(W1, b1, g1, be1, m1, v1)
    W2p, B2 = _fold_bn(W2, b2, g2, be2, m2, v2)
    W3p, B3 = _fold_bn(W3, b3, g3, be3, m3, v3)

    def bc(v, reps):
        return np.ascontiguousarray(
            np.tile(np.asarray(v, np.float32)[None, :], (P, reps)))

    shared = {
        "W1p": W1p, "W2p": W2p, "W3p": W3p,
        "B1bc": bc(B1, SUPER), "B2bc": bc(B2, SUPER), "B3bc": bc(B3, SUPER),
        "fw1": np.asarray(fw1, np.float32), "fw2": np.asarray(fw2, np.float32),
        "fb1bc": bc(fb1, 1), "fb2bc": bc(fb2, 1),
        "cinv": cinv.reshape(B, 1),
    }

    in_maps = []
    for c in range(M):
        im = {
            "x_sh": xs[c],
            "dinv_bc": dinv_bc[c],
            "batch_sh": batch_sh[c].reshape(NSP, 1),
            "ell_idx": ell[c],
        }
        im.update(shared)
        in_maps.append(im)

    meta = dict(NS=NS, NSP=NSP, T=T, GSUP=GSUP, NT=NT,
                Kg=tuple(int(k) for k in Kg), TOT=TOT)
    return in_maps, meta


def build_program(meta):
    NSP, T, GSUP, NT = meta["NSP"], meta["T"], meta["GSUP"], meta["NT"]
    Kg, TOT = meta["Kg"], meta["TOT"]
    goff = [0]
    for k in Kg:
        goff.append(goff[-1] + P * SUPER * k)

    nc = bacc.Bacc("TRN2", target_bir_lowering=False, debug=False,
                   num_devices=M)

    x_sh = nc.dram_tensor("x_sh", [NSP, F_IN], F32, kind="ExternalInput")
    dinv_bc_d = nc.dram_tensor("dinv_bc", [P, T * H], F32, kind="ExternalInput")
    batch_sh = nc.dram_tensor("batch_sh", [NSP, 1], I32, kind="ExternalInput")
    ell_idx = nc.dram_tensor("ell_idx", [TOT], I32, kind="ExternalInput")
    W1p = nc.dram_tensor("W1p", [F_IN, H], F32, kind="ExternalInput")
    W2p = nc.dram_tensor("W2p", [H, H], F32, kind="ExternalInput")
    W3p = nc.dram_tensor("W3p", [H, H], F32, kind="ExternalInput")
    B1bc = nc.dram_tensor("B1bc", [P, SUPER * H], F32, kind="ExternalInput")
    B2bc = nc.dram_tensor("B2bc", [P, SUPER * H], F32, kind="ExternalInput")
    B3bc = nc.dram_tensor("B3bc", [P, SUPER * H], F32, kind="ExternalInput")
    fw1_d = nc.dram_tensor("fw1", [H, H // 2], F32, kind="ExternalInput")
    fw2_d = nc.dram_tensor("fw2", [H // 2, C], F32, kind="ExternalInput")
    fb1bc = nc.dram_tensor("fb1bc", [P, H // 2], F32, kind="ExternalInput")
    fb2bc = nc.dram_tensor("fb2bc", [P, C], F32, kind="ExternalInput")
    cinv_d = nc.dram_tensor("cinv", [B, 1], F32, kind="ExternalInput")
    out_d = nc.dram_tensor("out", [B, C], F32, kind="ExternalOutput")

    vshard = nc.dram_tensor("vshard", [NSP, H], F32)
    Vt = [nc.dram_tensor(f"Vt{l}", [NT, H], F32) for l in range(3)]
    pool_in = nc.dram_tensor("pool_in", [B, H], F32)
    pool_out = nc.dram_tensor("pool_out", [B, H], F32)

    groups = [list(range(M))]
    SW = SUPER * H

    with tile.TileContext(nc) as tc:
        with (
            tc.tile_pool(name="resident", bufs=1) as rp,
            tc.tile_pool(name="work", bufs=2) as wp,
            tc.tile_pool(name="gather", bufs=2) as gp,
            tc.tile_pool(name="psum", bufs=2, space="PSUM") as pp,
            tc.tile_pool(name="psum_acc", bufs=1, space="PSUM") as pacc,
        ):
            ident = rp.tile([P, P], F32, tag="ident")
            make_identity(nc, ident[:])
            iota_i = rp.tile([P, P], I32, tag="iota_i")
            nc.gpsimd.iota(iota_i[:], pattern=[[1, P]], base=0,
                           channel_multiplier=0)
            iota_f = rp.tile([P, P], F32, tag="iota_f")
            nc.vector.tensor_copy(iota_f[:], iota_i[:])

            dinvbc = rp.tile([P, T * H], F32, tag="dinvbc")
            nc.sync.dma_start(out=dinvbc[:], in_=dinv_bc_d[:, :])
            selfb = rp.tile([P, T * H], F32, tag="selfb")
            r3b = rp.tile([P, T * H], F32, tag="r3b")

            w1s = rp.tile([F_IN, H], F32, tag="w1s")
            nc.sync.dma_start(out=w1s[:], in_=W1p[:, :])
            w2s = rp.tile([H, H], F32, tag="w2s")
            nc.sync.dma_start(out=w2s[:], in_=W2p[:, :])
            w3s = rp.tile([H, H], F32, tag="w3s")
            nc.sync.dma_start(out=w3s[:], in_=W3p[:, :])
            bbc = []
            for l, src_t in enumerate((B1bc, B2bc, B3bc)):
                t_ = rp.tile([P, SW], F32, tag=f"bbc{l}")
                nc.sync.dma_start(out=t_[:], in_=src_t[:, :])
                bbc.append(t_)
            fw1s = rp.tile([H, H // 2], F32, tag="fw1s")
            nc.sync.dma_start(out=fw1s[:], in_=fw1_d[:, :])
            fw2s = rp.tile([H // 2, C], F32, tag="fw2s")
            nc.sync.dma_start(out=fw2s[:], in_=fw2_d[:, :])
            fb1s = rp.tile([P, H // 2], F32, tag="fb1s")
            nc.sync.dma_start(out=fb1s[:], in_=fb1bc[:, :])
            fb2s = rp.tile([P, C], F32, tag="fb2s")
            nc.sync.dma_start(out=fb2s[:], in_=fb2bc[:, :])
            cinvs = rp.tile([B, 1], F32, tag="cinvs")
            nc.sync.dma_start(out=cinvs[:], in_=cinv_d[:, :])

            def store_super(dram, sbuf_ap, g):
                rows = dram[g * SUPER * P:(g + 1) * SUPER * P, :]
                dview = rows.rearrange("(t p) j -> p t j", p=P)
                sview = sbuf_ap.rearrange("p (t j) -> p t j", j=H)
                nc.sync.dma_start(out=dview, in_=sview)

            # ---------------- M1: u1 = dinv * (x @ W1') ----------------
            for g in range(GSUP):
                xt = wp.tile([P, SUPER * F_IN], F32, tag="xt")
                rows = x_sh[g * SUPER * P:(g + 1) * SUPER * P, :]
                nc.sync.dma_start(
                    out=xt[:].rearrange("p (t j) -> p t j", j=F_IN),
                    in_=rows.rearrange("(t p) j -> p t j", p=P))
                xd = wp.tile([P, SUPER * F_IN], F32, tag="xd")
                for b_ in range(SUPER):
                    t = g * SUPER + b_
                    nc.vector.tensor_tensor(
                        out=xd[:, b_ * F_IN:(b_ + 1) * F_IN],
                        in0=xt[:, b_ * F_IN:(b_ + 1) * F_IN],
                        in1=dinvbc[:, t * H:t * H + F_IN],
                        op=ALU.mult)
                tr_ps = pp.tile([F_IN, SUPER * P], F32, tag="tr_ps")
                for b_ in range(SUPER):
                    nc.tensor.transpose(
                        out=tr_ps[:, b_ * P:(b_ + 1) * P],
                        in_=xd[:, b_ * F_IN:(b_ + 1) * F_IN],
                        identity=ident[:])
                xdT = wp.tile([F_IN, SUPER * P], F32, tag="xdT")
                nc.vector.tensor_copy(xdT[:], tr_ps[:])
                z_ps = pp.tile([P, SW], F32, tag="z_ps")
                for b_ in range(SUPER):
                    nc.tensor.matmul(
                        out=z_ps[:, b_ * H:(b_ + 1) * H],
                        lhsT=xdT[:, b_ * P:(b_ + 1) * P],
                        rhs=w1s[:], start=True, stop=True)
                nc.vector.tensor_copy(selfb[:, g * SW:(g + 1) * SW], z_ps[:])
                store_super(vshard, selfb[:, g * SW:(g + 1) * SW], g)

            nc.gpsimd.collective_compute(
                "AllGather", ALU.bypass, replica_groups=groups,
                ins=[vshard.ap().opt()], outs=[Vt[0].ap().opt()])

            # ---------------- Layers (gather phases) ----------------
            for l in range(3):
                table = Vt[l]
                for g in range(GSUP):
                    K = Kg[g]
                    gslice = slice(g * SW, (g + 1) * SW)
                    if K > 0:
                        flat = ell_idx[goff[g]:goff[g + 1]]
                        flat2 = flat.rearrange("(p s) -> p s", p=P)
                        nchunks = math.ceil(K / K_CHUNK)
                        for ci in range(nchunks):
                            c0 = ci * K_CHUNK
                            kc = min(K_CHUNK, K - c0)
                            sw_c = kc * SUPER
                            idx_t = wp.tile([P, K_CHUNK * SUPER], I32,
                                            tag="idx")
                            nc.sync.dma_start(
                                out=idx_t[:, :sw_c],
                                in_=flat2[:, c0 * SUPER:(c0 + kc) * SUPER])
                            gbuf = gp.tile([P, K_CHUNK * SUPER * H], F32,
                                           tag="gbuf")
                            # Walrus honors ONE offset per partition per
                            # indirect DMA (probed on HW): one op per slot.
                            for s in range(sw_c):
                                nc.gpsimd.indirect_dma_start(
                                    out=gbuf[:, s * H:(s + 1) * H],
                                    out_offset=None,
                                    in_=table[:, :],
                                    in_offset=IndirectOffsetOnAxis(
                                        ap=idx_t[:, s:s + 1], axis=0))
                            cur = kc
                            while cur > 1:
                                if cur % 2 == 1:
                                    nc.vector.tensor_add(
                                        gbuf[:, :SW], gbuf[:, :SW],
                                        gbuf[:, (cur - 1) * SW:cur * SW])
                                    cur -= 1
                                    if cur == 1:
                                        break
                                half = cur // 2
                                nc.vector.tensor_add(
                                    gbuf[:, :half * SW],
                                    gbuf[:, :half * SW],
                                    gbuf[:, half * SW:2 * half * SW])
                                cur = half
                            nc.vector.tensor_add(
                                selfb[:, gslice], selfb[:, gslice],
                                gbuf[:, :SW])

                    if l == 0:
                        zsrc = selfb[:, gslice]
                    else:
                        w_s = w2s if l == 1 else w3s
                        tr_ps = pp.tile([H, SUPER * P], F32, tag="tr_ps")
                        for b_ in range(SUPER):
                            t = g * SUPER + b_
                            nc.tensor.transpose(
                                out=tr_ps[:, b_ * P:(b_ + 1) * P],
                                in_=selfb[:, t * H:(t + 1) * H],
                                identity=ident[:])
                        aggvT = wp.tile([H, SUPER * P], F32, tag="aggvT")
                        nc.vector.tensor_copy(aggvT[:], tr_ps[:])
                        z_ps = pp.tile([P, SW], F32, tag="z_ps")
                        for b_ in range(SUPER):
                            nc.tensor.matmul(
                                out=z_ps[:, b_ * H:(b_ + 1) * H],
                                lhsT=aggvT[:, b_ * P:(b_ + 1) * P],
                                rhs=w_s[:], start=True, stop=True)
                        zsrc = z_ps[:]

                    if l < 2:
                        rt_tile = wp.tile([P, SW], F32, tag="rt")
                        rt = rt_tile[:]
                    else:
                        rt = r3b[:, gslice]
                    nc.vector.tensor_tensor(out=rt, in0=zsrc,
                                            in1=dinvbc[:, gslice],
                                            op=ALU.mult)
                    nc.vector.tensor_add(rt, rt, bbc[l][:])
                    nc.vector.tensor_scalar_max(rt, rt, 0.0)

                    if l < 2:
                        nc.vector.tensor_tensor(out=selfb[:, gslice], in0=rt,
                                                in1=dinvbc[:, gslice],
                                                op=ALU.mult)
                        store_super(vshard, selfb[:, gslice], g)

                if l < 2:
                    nc.gpsimd.collective_compute(
                        "AllGather", ALU.bypass, replica_groups=groups,
                        ins=[vshard.ap().opt()], outs=[Vt[l + 1].ap().opt()])

            # ---------------- Global mean pool ----------------
            pool_ps = pacc.tile([B, H], F32, tag="pool_ps")
            for t in range(T):
                bt = wp.tile([P, 1], I32, tag="bt")
                nc.sync.dma_start(out=bt[:], in_=batch_sh[t * P:(t + 1) * P, :])
                btf = wp.tile([P, 1], F32, tag="btf")
                nc.vector.tensor_copy(btf[:], bt[:])
                S = wp.tile([P, P], F32, tag="S")
                nc.vector.tensor_scalar(S[:], iota_f[:], btf[:], None,
                                        ALU.is_equal)
                nc.tensor.matmul(out=pool_ps[:], lhsT=S[:],
                                 rhs=r3b[:, t * H:(t + 1) * H],
                                 start=(t == 0), stop=(t == T - 1))
            pool_sb = wp.tile([B, H], F32, tag="pool_sb")
            nc.vector.tensor_copy(pool_sb[:], pool_ps[:])
            nc.sync.dma_start(out=pool_in[:, :], in_=pool_sb[:])
            nc.gpsimd.collective_compute(
                "AllReduce", ALU.add, replica_groups=groups,
                ins=[pool_in.ap().opt()], outs=[pool_out.ap().opt()])

            pooled = wp.tile([B, H], F32, tag="pooled")
            nc.sync.dma_start(out=pooled[:], in_=pool_out[:, :])
            nc.vector.tensor_scalar(pooled[:], pooled[:], cinvs[:], None,
                                    ALU.mult)

            # ---------------- MLP head ----------------
            trp = pp.tile([H, B], F32, tag="mlp_ps")
            nc.tensor.transpose(out=trp[:], in_=pooled[:], identity=ident[:])
            pT = wp.tile([H, B], F32, tag="pT")
            nc.vector.tensor_copy(pT[:], trp[:])
            h1ps = pp.tile([B, H // 2], F32, tag="mlp_ps")
            nc.tensor.matmul(out=h1ps[:], lhsT=pT[:], rhs=fw1s[:],
                             start=True, stop=True)
            h1 = wp.tile([B, H // 2], F32, tag="h1")
            nc.vector.tensor_add(h1[:], h1ps[:], fb1s[:])
            nc.vector.tensor_scalar_max(h1[:], h1[:], 0.0)
            tr2 = pp.tile([H // 2, B], F32, tag="mlp_ps")
            nc.tensor.transpose(out=tr2[:], in_=h1[:], identity=ident[:])
            h1T = wp.tile([H // 2, B], F32, tag="h1T")
            nc.vector.tensor_copy(h1T[:], tr2[:])
            o_ps = pp.tile([B, C], F32, tag="mlp_ps")
            nc.tensor.matmul(out=o_ps[:], lhsT=h1T[:], rhs=fw2s[:],
                             start=True, stop=True)
            o_sb = wp.tile([B, C], F32, tag="o_sb")
            nc.vector.tensor_add(o_sb[:], o_ps[:], fb2s[:])
            nc.sync.dma_start(out=out_d[:, :], in_=o_sb[:])

    nc.compile()
    return nc


_CACHE: dict = {}


def kernel(**inputs) -> np.ndarray:
    global LAST_EXEC_NS
    in_maps, meta = preprocess(**inputs)
    key = (meta["Kg"], meta["TOT"])
    nc = _CACHE.get(key)
    if nc is None:
        nc = build_program(meta)
        _CACHE[key] = nc
    res = bass_utils.run_bass_kernel_spmd(
        nc, in_maps, core_ids=list(range(M)), trace=TRACE)
    LAST_EXEC_NS = res.exec_time_ns
    return np.asarray(res.results[0]["out"])



# revision 3
# speedup vs baseline: 17.7910x; 17.7910x over previous
"""Trainium2 Bass kernel for CascadeClassifierGNN (3-layer GCN + BN + ReLU,
global mean pool, 2-layer MLP head), sharded across 8 NeuronCores.

Sharding strategy: nodes and their incident (by-destination) edges are
partitioned across the 8 cores; node ids are relabeled per-core by in-degree
so the per-destination ELL gather structure has minimal padding. Per layer,
each core computes its shard of the gather table v = dinv * r (layer 1:
u1 = dinv * (x @ W1')), an AllGather replicates the full table into every
core's HBM, and each core runs indirect-DMA gathers (128 rows x 256B per op)
+ DVE tree reductions to form neighborhood sums for its own destinations.
BatchNorm is folded into the layer weights on the host (W' = W * A, bias B).
The global mean pool uses one-hot selection matmuls accumulated in PSUM
followed by a tiny AllReduce; the MLP head runs replicated on every core.
"""

import math
import os

import numpy as np

import concourse.bacc as bacc
import concourse.mybir as mybir
import concourse.tile as tile
from concourse import bass_utils
from concourse.bass import IndirectOffsetOnAxis
from concourse.masks import make_identity

F32 = mybir.dt.float32
I32 = mybir.dt.int32
ALU = mybir.AluOpType

# Problem configuration (hardcoded per contest contract).
N = 100000
E = 1600000
F_IN = 10
H = 64
B = 128
C = 3
EPS = 1e-5
M = 8           # cores
P = 128         # partitions
SUPER = 4       # dst tiles per super-tile (gather/reduce granularity)
K_CHUNK = 32    # max ELL slots gathered per chunk

TRACE = os.environ.get("GNN_TRACE", "0") == "1"
SKIP_GATHER = os.environ.get("GNN_SKIP_GATHER", "0") == "1"
SKIP_COLL = os.environ.get("GNN_SKIP_COLL", "0") == "1"
LAST_EXEC_NS = None


def _fold_bn(Wl, bl, gl, bel, ml, vl):
    A = (np.asarray(gl, np.float32)
         / np.sqrt(np.asarray(vl, np.float32) + np.float32(EPS)))
    Wp = (np.asarray(Wl, np.float32) * A[None, :]).astype(np.float32)
    Bv = ((np.asarray(bl, np.float32) - np.asarray(ml, np.float32)) * A
          + np.asarray(bel, np.float32)).astype(np.float32)
    return Wp, Bv


def preprocess(x, edge_index, batch, W1, b1, g1, be1, m1, v1,
               W2, b2, g2, be2, m2, v2, W3, b3, g3, be3, m3, v3,
               fw1, fb1, fw2, fb2):
    x = np.asarray(x, dtype=np.float32)
    src = np.asarray(edge_index[0], dtype=np.int64)
    dst = np.asarray(edge_index[1], dtype=np.int64)
    batch = np.asarray(batch, dtype=np.int64)

    assert N % M == 0
    NS = N // M
    T = math.ceil((NS + 1) / (P * SUPER)) * SUPER
    NSP = T * P
    GSUP = T // SUPER
    NT = M * NSP

    deg = (np.bincount(dst, minlength=N) + 1.0).astype(np.float32)
    dinv = (1.0 / np.sqrt(deg)).astype(np.float32)

    relabel = np.empty(N, dtype=np.int64)
    perms = []
    for c in range(M):
        lo = c * NS
        order = np.argsort(-(deg[lo:lo + NS]), kind="stable")
        perms.append(order)
        relabel[lo + order] = c * NSP + np.arange(NS)
    new_src = relabel[src].astype(np.int32)

    owner = dst // NS
    local_rank = relabel[dst] - owner * NSP

    per_core = []
    Kg = np.zeros(GSUP, dtype=np.int64)
    for c in range(M):
        mask = owner == c
        e_rank = local_rank[mask]
        e_src = new_src[mask]
        cnt = np.bincount(e_rank, minlength=NSP)
        per_core.append((e_rank, e_src, cnt))
        cg = cnt.reshape(GSUP, SUPER * P).max(axis=1)
        Kg = np.maximum(Kg, cg)
    Kg = Kg.astype(np.int64)

    goff = np.zeros(GSUP + 1, dtype=np.int64)
    goff[1:] = np.cumsum(P * SUPER * Kg)
    TOT = int(goff[-1])

    dead_rows = (np.arange(M) * NSP + NSP - 1).astype(np.int32)

    ell = np.empty((M, TOT), dtype=np.int32)
    for c in range(M):
        e_rank, e_src, cnt = per_core[c]
        arr = np.full(TOT, dead_rows[c], dtype=np.int32)
        order = np.argsort(e_rank, kind="stable")
        er = e_rank[order]
        es = e_src[order]
        start = np.zeros(NSP, dtype=np.int64)
        start[1:] = np.cumsum(cnt)[:-1]
        slot = np.arange(er.shape[0], dtype=np.int64) - start[er]
        t = er // P
        p = er % P
        g = t // SUPER
        b_ = t % SUPER
        pos = goff[g] + p * (Kg[g] * SUPER) + slot * SUPER + b_
        arr[pos] = es
        ell[c] = arr

    xs = np.zeros((M, NSP, F_IN), dtype=np.float32)
    dinv_sh = np.zeros((M, NSP), dtype=np.float32)
    batch_sh = np.full((M, NSP), -1, dtype=np.int32)
    for c in range(M):
        lo = c * NS
        order = perms[c]
        xs[c, :NS] = x[lo + order]
        dinv_sh[c, :NS] = dinv[lo + order]
        batch_sh[c, :NS] = batch[lo + order]

    dv = dinv_sh.reshape(M, T, P).transpose(0, 2, 1)
    dinv_bc = np.repeat(dv[:, :, :, None], H, axis=3).reshape(M, P, T * H)
    dinv_bc = np.ascontiguousarray(dinv_bc, dtype=np.float32)

    counts = np.bincount(batch, minlength=B).astype(np.float32)
    cinv = (1.0 / np.maximum(counts, 1.0)).astype(np.float32)

    W1p, B1 = _fold_bn(W1, b1, g1, be1, m1, v1)
    W2p, B2 = _fold_bn(W2, b2, g2, be2, m2, v2)
    W3p, B3 = _fold_bn(W3, b3, g3, be3, m3, v3)

    def bc(v, reps):
        return np.ascontiguousarray(
            np.tile(np.asarray(v, np.float32)[None, :], (P, reps)))

    shared = {
        "W1p": W1p, "W2p": W2p, "W3p": W3p,
        "B1bc": bc(B1, SUPER), "B2bc": bc(B2, SUPER), "B3bc": bc(B3, SUPER),
        "fw1": np.asarray(fw1, np.float32), "fw2": np.asarray(fw2, np.float32),
        "fb1bc": bc(fb1, 1), "fb2bc": bc(fb2, 1),
        "cinv": cinv.reshape(B, 1),
    }

    in_maps = []
    for c in range(M):
        im = {
            "x_sh": xs[c],
            "dinv_bc": dinv_bc[c],
            "batch_sh": batch_sh[c].reshape(NSP, 1),
            "ell_idx": ell[c],
        }
        im.update(shared)
        in_maps.append(im)

    meta = dict(NS=NS, NSP=NSP, T=T, GSUP=GSUP, NT=NT,
                Kg=tuple(int(k) for k in Kg), TOT=TOT)
    return in_maps, meta


def build_program(meta):
    NSP, T, GSUP, NT = meta["NSP"], meta["T"], meta["GSUP"], meta["NT"]
    Kg, TOT = meta["Kg"], meta["TOT"]
    goff = [0]
    for k in Kg:
        goff.append(goff[-1] + P * SUPER * k)

    nc = bacc.Bacc("TRN2", target_bir_lowering=False, debug=False,
                   num_devices=M)

    x_sh = nc.dram_tensor("x_sh", [NSP, F_IN], F32, kind="ExternalInput")
    dinv_bc_d = nc.dram_tensor("dinv_bc", [P, T * H], F32, kind="ExternalInput")
    batch_sh = nc.dram_tensor("batch_sh", [NSP, 1], I32, kind="ExternalInput")
    ell_idx = nc.dram_tensor("ell_idx", [TOT], I32, kind="ExternalInput")
    W1p = nc.dram_tensor("W1p", [F_IN, H], F32, kind="ExternalInput")
    W2p = nc.dram_tensor("W2p", [H, H], F32, kind="ExternalInput")
    W3p = nc.dram_tensor("W3p", [H, H], F32, kind="ExternalInput")
    B1bc = nc.dram_tensor("B1bc", [P, SUPER * H], F32, kind="ExternalInput")
    B2bc = nc.dram_tensor("B2bc", [P, SUPER * H], F32, kind="ExternalInput")
    B3bc = nc.dram_tensor("B3bc", [P, SUPER * H], F32, kind="ExternalInput")
    fw1_d = nc.dram_tensor("fw1", [H, H // 2], F32, kind="ExternalInput")
    fw2_d = nc.dram_tensor("fw2", [H // 2, C], F32, kind="ExternalInput")
    fb1bc = nc.dram_tensor("fb1bc", [P, H // 2], F32, kind="ExternalInput")
    fb2bc = nc.dram_tensor("fb2bc", [P, C], F32, kind="ExternalInput")
    cinv_d = nc.dram_tensor("cinv", [B, 1], F32, kind="ExternalInput")
    out_d = nc.dram_tensor("out", [B, C], F32, kind="ExternalOutput")

    vshard = nc.dram_tensor("vshard", [NSP, H], F32)
    Vt = [nc.dram_tensor(f"Vt{l}", [NT, H], F32) for l in range(3)]
    pool_in = nc.dram_tensor("pool_in", [B, H], F32)
    pool_out = nc.dram_tensor("pool_out", [B, H], F32)

    groups = [list(range(M))]
    SW = SUPER * H

    with tile.TileContext(nc) as tc:
        with (
            tc.tile_pool(name="resident", bufs=1) as rp,
            tc.tile_pool(name="work", bufs=2) as wp,
            tc.tile_pool(name="gather", bufs=2) as gp,
            tc.tile_pool(name="psum", bufs=2, space="PSUM") as pp,
            tc.tile_pool(name="psum_acc", bufs=1, space="PSUM") as pacc,
        ):
            ident = rp.tile([P, P], F32, tag="ident")
            make_identity(nc, ident[:])
            iota_i = rp.tile([P, P], I32, tag="iota_i")
            nc.gpsimd.iota(iota_i[:], pattern=[[1, P]], base=0,
                           channel_multiplier=0)
            iota_f = rp.tile([P, P], F32, tag="iota_f")
            nc.vector.tensor_copy(iota_f[:], iota_i[:])

            dinvbc = rp.tile([P, T * H], F32, tag="dinvbc")
            nc.sync.dma_start(out=dinvbc[:], in_=dinv_bc_d[:, :])
            selfb = rp.tile([P, T * H], F32, tag="selfb")
            r3b = rp.tile([P, T * H], F32, tag="r3b")

            w1s = rp.tile([F_IN, H], F32, tag="w1s")
            nc.sync.dma_start(out=w1s[:], in_=W1p[:, :])
            w2s = rp.tile([H, H], F32, tag="w2s")
            nc.sync.dma_start(out=w2s[:], in_=W2p[:, :])
            w3s = rp.tile([H, H], F32, tag="w3s")
            nc.sync.dma_start(out=w3s[:], in_=W3p[:, :])
            bbc = []
            for l, src_t in enumerate((B1bc, B2bc, B3bc)):
                t_ = rp.tile([P, SW], F32, tag=f"bbc{l}")
                nc.sync.dma_start(out=t_[:], in_=src_t[:, :])
                bbc.append(t_)
            fw1s = rp.tile([H, H // 2], F32, tag="fw1s")
            nc.sync.dma_start(out=fw1s[:], in_=fw1_d[:, :])
            fw2s = rp.tile([H // 2, C], F32, tag="fw2s")
            nc.sync.dma_start(out=fw2s[:], in_=fw2_d[:, :])
            fb1s = rp.tile([P, H // 2], F32, tag="fb1s")
            nc.sync.dma_start(out=fb1s[:], in_=fb1bc[:, :])
            fb2s = rp.tile([P, C], F32, tag="fb2s")
            nc.sync.dma_start(out=fb2s[:], in_=fb2bc[:, :])
            cinvs = rp.tile([B, 1], F32, tag="cinvs")
            nc.sync.dma_start(out=cinvs[:], in_=cinv_d[:, :])

            def store_super(dram, sbuf_ap, g):
                rows = dram[g * SUPER * P:(g + 1) * SUPER * P, :]
                dview = rows.rearrange("(t p) j -> p t j", p=P)
                sview = sbuf_ap.rearrange("p (t j) -> p t j", j=H)
                nc.sync.dma_start(out=dview, in_=sview)

            # ---------------- M1: u1 = dinv * (x @ W1') ----------------
            for g in range(GSUP):
                xt = wp.tile([P, SUPER * F_IN], F32, tag="xt")
                rows = x_sh[g * SUPER * P:(g + 1) * SUPER * P, :]
                nc.sync.dma_start(
                    out=xt[:].rearrange("p (t j) -> p t j", j=F_IN),
                    in_=rows.rearrange("(t p) j -> p t j", p=P))
                xd = wp.tile([P, SUPER * F_IN], F32, tag="xd")
                for b_ in range(SUPER):
                    t = g * SUPER + b_
                    nc.vector.tensor_tensor(
                        out=xd[:, b_ * F_IN:(b_ + 1) * F_IN],
                        in0=xt[:, b_ * F_IN:(b_ + 1) * F_IN],
                        in1=dinvbc[:, t * H:t * H + F_IN],
                        op=ALU.mult)
                tr_ps = pp.tile([F_IN, SUPER * P], F32, tag="tr_ps")
                for b_ in range(SUPER):
                    nc.tensor.transpose(
                        out=tr_ps[:, b_ * P:(b_ + 1) * P],
                        in_=xd[:, b_ * F_IN:(b_ + 1) * F_IN],
                        identity=ident[:])
                xdT = wp.tile([F_IN, SUPER * P], F32, tag="xdT")
                nc.vector.tensor_copy(xdT[:], tr_ps[:])
                z_ps = pp.tile([P, SW], F32, tag="z_ps")
                for b_ in range(SUPER):
                    nc.tensor.matmul(
                        out=z_ps[:, b_ * H:(b_ + 1) * H],
                        lhsT=xdT[:, b_ * P:(b_ + 1) * P],
                        rhs=w1s[:], start=True, stop=True)
                nc.vector.tensor_copy(selfb[:, g * SW:(g + 1) * SW], z_ps[:])
                store_super(vshard, selfb[:, g * SW:(g + 1) * SW], g)

            if not SKIP_COLL:
                nc.gpsimd.collective_compute(
                    "AllGather", ALU.bypass, replica_groups=groups,
                    ins=[vshard.ap().opt()], outs=[Vt[0].ap().opt()])

            # ---------------- Layers (gather phases) ----------------
            for l in range(3):
                table = Vt[l]
                for g in range(GSUP):
                    K = Kg[g]
                    gslice = slice(g * SW, (g + 1) * SW)
                    if K > 0:
                        flat = ell_idx[goff[g]:goff[g + 1]]
                        flat2 = flat.rearrange("(p s) -> p s", p=P)
                        nchunks = math.ceil(K / K_CHUNK)
                        for ci in range(nchunks):
                            c0 = ci * K_CHUNK
                            kc = min(K_CHUNK, K - c0)
                            sw_c = kc * SUPER
                            idx_t = wp.tile([P, K_CHUNK * SUPER], I32,
                                            tag="idx")
                            nc.sync.dma_start(
                                out=idx_t[:, :sw_c],
                                in_=flat2[:, c0 * SUPER:(c0 + kc) * SUPER])
                            gbuf = gp.tile([P, K_CHUNK * SUPER * H], F32,
                                           tag="gbuf")
                            # Walrus honors ONE offset per partition per
                            # indirect DMA (probed on HW): one op per slot.
                            for s in range(sw_c if not SKIP_GATHER else 0):
                                nc.gpsimd.indirect_dma_start(
                                    out=gbuf[:, s * H:(s + 1) * H],
                                    out_offset=None,
                                    in_=table[:, :],
                                    in_offset=IndirectOffsetOnAxis(
                                        ap=idx_t[:, s:s + 1], axis=0))
                            cur = kc
                            while cur > 1:
                                if cur % 2 == 1:
                                    nc.vector.tensor_add(
                                        gbuf[:, :SW], gbuf[:, :SW],
                                        gbuf[:, (cur - 1) * SW:cur * SW])
                                    cur -= 1
                                    if cur == 1:
                                        break
                                half = cur // 2
                                nc.vector.tensor_add(
                                    gbuf[:, :half * SW],
                                    gbuf[:, :half * SW],
                                    gbuf[:, half * SW:2 * half * SW])
                                cur = half
                            nc.vector.tensor_add(
                                selfb[:, gslice], selfb[:, gslice],
                                gbuf[:, :SW])

                    if l == 0:
                        zsrc = selfb[:, gslice]
                    else:
                        w_s = w2s if l == 1 else w3s
                        tr_ps = pp.tile([H, SUPER * P], F32, tag="tr_ps")
                        for b_ in range(SUPER):
                            t = g * SUPER + b_
                            nc.tensor.transpose(
                                out=tr_ps[:, b_ * P:(b_ + 1) * P],
                                in_=selfb[:, t * H:(t + 1) * H],
                                identity=ident[:])
                        aggvT = wp.tile([H, SUPER * P], F32, tag="aggvT")
                        nc.vector.tensor_copy(aggvT[:], tr_ps[:])
                        z_ps = pp.tile([P, SW], F32, tag="z_ps")
                        for b_ in range(SUPER):
                            nc.tensor.matmul(
                                out=z_ps[:, b_ * H:(b_ + 1) * H],
                                lhsT=aggvT[:, b_ * P:(b_ + 1) * P],
                                rhs=w_s[:], start=True, stop=True)
                        zsrc = z_ps[:]

                    if l < 2:
                        rt_tile = wp.tile([P, SW], F32, tag="rt")
                        rt = rt_tile[:]
                    else:
                        rt = r3b[:, gslice]
                    nc.vector.tensor_tensor(out=rt, in0=zsrc,
                                            in1=dinvbc[:, gslice],
                                            op=ALU.mult)
                    nc.vector.tensor_add(rt, rt, bbc[l][:])
                    nc.vector.tensor_scalar_max(rt, rt, 0.0)

                    if l < 2:
                        nc.vector.tensor_tensor(out=selfb[:, gslice], in0=rt,
                                                in1=dinvbc[:, gslice],
                                                op=ALU.mult)
                        store_super(vshard, selfb[:, gslice], g)

                if l < 2 and not SKIP_COLL:
                    nc.gpsimd.collective_compute(
                        "AllGather", ALU.bypass, replica_groups=groups,
                        ins=[vshard.ap().opt()], outs=[Vt[l + 1].ap().opt()])

            # ---------------- Global mean pool ----------------
            pool_ps = pacc.tile([B, H], F32, tag="pool_ps")
            for t in range(T):
                bt = wp.tile([P, 1], I32, tag="bt")
                nc.sync.dma_start(out=bt[:], in_=batch_sh[t * P:(t + 1) * P, :])
                btf = wp.tile([P, 1], F32, tag="btf")
                nc.vector.tensor_copy(btf[:], bt[:])
                S = wp.tile([P, P], F32, tag="S")
                nc.vector.tensor_scalar(S[:], iota_f[:], btf[:], None,
                                        ALU.is_equal)
                nc.tensor.matmul(out=pool_ps[:], lhsT=S[:],
                                 rhs=r3b[:, t * H:(t + 1) * H],
                                 start=(t == 0), stop=(t == T - 1))
            pool_sb = wp.tile([B, H], F32, tag="pool_sb")
            nc.vector.tensor_copy(pool_sb[:], pool_ps[:])
            nc.sync.dma_start(out=pool_in[:, :], in_=pool_sb[:])
            nc.gpsimd.collective_compute(
                "AllReduce", ALU.add, replica_groups=groups,
                ins=[pool_in.ap().opt()], outs=[pool_out.ap().opt()])

            pooled = wp.tile([B, H], F32, tag="pooled")
            nc.sync.dma_start(out=pooled[:], in_=pool_out[:, :])
            nc.vector.tensor_scalar(pooled[:], pooled[:], cinvs[:], None,
                                    ALU.mult)

            # ---------------- MLP head ----------------
            trp = pp.tile([H, B], F32, tag="mlp_ps")
            nc.tensor.transpose(out=trp[:], in_=pooled[:], identity=ident[:])
            pT = wp.tile([H, B], F32, tag="pT")
            nc.vector.tensor_copy(pT[:], trp[:])
            h1ps = pp.tile([B, H // 2], F32, tag="mlp_ps")
            nc.tensor.matmul(out=h1ps[:], lhsT=pT[:], rhs=fw1s[:],
                             start=True, stop=True)
            h1 = wp.tile([B, H // 2], F32, tag="h1")
            nc.vector.tensor_add(h1[:], h1ps[:], fb1s[:])
            nc.vector.tensor_scalar_max(h1[:], h1[:], 0.0)
            tr2 = pp.tile([H // 2, B], F32, tag="mlp_ps")
            nc.tensor.transpose(out=tr2[:], in_=h1[:], identity=ident[:])
            h1T = wp.tile([H // 2, B], F32, tag="h1T")
            nc.vector.tensor_copy(h1T[:], tr2[:])
            o_ps = pp.tile([B, C], F32, tag="mlp_ps")
            nc.tensor.matmul(out=o_ps[:], lhsT=h1T[:], rhs=fw2s[:],
                             start=True, stop=True)
            o_sb = wp.tile([B, C], F32, tag="o_sb")
            nc.vector.tensor_add(o_sb[:], o_ps[:], fb2s[:])
            nc.sync.dma_start(out=out_d[:, :], in_=o_sb[:])

    nc.compile()
    return nc


_CACHE: dict = {}


def kernel(**inputs) -> np.ndarray:
    global LAST_EXEC_NS
    in_maps, meta = preprocess(**inputs)
    key = (meta["Kg"], meta["TOT"])
    nc = _CACHE.get(key)
    if nc is None:
        nc = build_program(meta)
        _CACHE[key] = nc
    res = bass_utils.run_bass_kernel_spmd(
        nc, in_maps, core_ids=list(range(M)), trace=TRACE)
    LAST_EXEC_NS = res.exec_time_ns
    return np.asarray(res.results[0]["out"])

